# revision 33
# baseline (speedup 1.0000x reference)
"""Trainium2 Bass kernel for y = 2*(einsum('bct,oc->bot', pre, W_pre) + b_pre).

Shapes (hardcoded): pre [16, 512, 4096] f32, W_pre [512, 512] f32, b_pre [512] f32.
Sharding: data-parallel over B across 8 cores (2 batches per core).

Per core: out[b, o, t] = sum_c W[o,c]*(2*pre)[b,c,t] + 2*bias[o] for 2 batches
(the reference's y+y is folded into x and bias on the host).

Precision/dtype plan (tolerance is max-rel-err < 2e-2 vs max|y|):
- moving operand x = fp8 E3M4 (float8e3): 4 mantissa bits. Host quantizes
  2*pre; max|2*pre| ~ 10.8 < 15.5 (e3m4 max normal), and the PE handles
  e3m4 subnormals exactly (verified by canary). Measured end-to-end rel
  err 1.61e-2. fp8e4 (needed for DoubleRow 2x PE) measures 4.2e-2 - fails
  - so the PE runs at the 1 row/cycle fp16-class rate and the kernel is
  PE-streaming-bound at ~55.3 us warm (131072 stream cycles @ 2.4 GHz).
- stationary W = fp16 (full precision for this data), output = fp16.
- fp8 x halves input DMA to 4.2 MB/core (out 8.4 MB fp16, W 0.5 MB):
  total ~13 MB << PE time at ~400 GB/s, so DMA fully hides.

Schedule notes (from NTFF traces):
- Engines can't start until their instruction iram loads (~53 KB @ ~16
  GB/s, done ~5.7 us) plus per-engine TENSOR_LOAD barriers; the scalar
  (ACT) engine is free first (~5.85 us), sync last (~6.6 us).
- SDMA round-robins *per packet* across active queues, so splitting the
  head loads over several queues delays every transfer's completion
  (measured: first matmul slipped 7 us). Instead the WHOLE input rides
  the scalar HWDGE queue in exact consumption order:
  W0, x b0c0, bias, W1, W2, W3, b0c1..c4, b1c0, b1c1 - and nothing else
  touches SDMA until the input is done (~20.5 us). Descriptor generation
  (~670 ns each) occupies the scalar engine until ~13.9 us, which is why
  ALL PSUM evictions run on DVE (66% busy warm - fits); scalar only
  desc-gens.
- PE HAM clock gate: 1.2 GHz cold, 2.4 GHz after ~3.4 us of sustained
  matmul activity. 8 dummy matmuls (N=256, scratch SBUF, own PSUM tag)
  bridge from ~6.2 us (memset lands on the gpsimd queue right after its
  preamble) to first-data at ~7.9 us.
- Matmul windows are >=256 columns: at N=128 the 107 ns LDWEIGHTS no
  longer hides behind the 56 ns stream and the PE becomes LDW-bound.
  Window plan per batch: b0 [256, 512x7, 256], b1 [512x7, 256, 256]
  (small first window so the first chunk lands early; small last
  windows so the final stores chase the last matmul closely).
- Output store groups ride the gpsimd SWDGE queue except b1's
  next-to-last (w7, 256 cols, on scalar - keeps sync's HWDGE queue
  empty) and the final w8 (256 cols), stored per-M-tile on
  sync/scalar/gpsimd/sync in parallel, each chasing its own DVE
  eviction, so the last HBM write is a 64 KB transfer ~1 us after the
  last matmul.
"""

import os
import sys

for _p in ("/opt/trn_rl_repo", "/root/.axon_site/_ro/trn_rl_repo"):
    if os.path.isdir(_p) and _p not in sys.path:
        sys.path.append(_p)

from contextlib import ExitStack

import ml_dtypes
import numpy as np

import concourse.bass as bass
import concourse.tile as tile
from concourse import bacc, mybir
from concourse.bass_utils import run_bass_kernel_spmd

B, C, T = 16, 512, 4096  # batch, channels (in == out), sequence
NCORES = 8
BPC = B // NCORES  # batches per core
P = 128
KT = C // P  # contraction tiles
MT = C // P  # output-channel tiles

X_DT = mybir.dt.float8e3  # moving operand + input DMA dtype (E3M4)
W_DT = mybir.dt.float16  # stationary operand
OUT_DT = mybir.dt.float16  # output SBUF + DRAM dtype
X_NP = ml_dtypes.float8_e3m4

# Input DMA chunks per batch (each chunk is one contiguous-line DMA).
# b0's chunk sizes track the SDMA ramp (~50 -> 420 GB/s over 9-15 us) so
# each chunk's completion stays ~1 us ahead of the PE's demand for it
# even with the observed +-1 us run-to-run ramp variance.
CHUNKS = {0: [256, 256, 256, 512, 512, 1024, 1280], 1: [2048, 2048]}
# Matmul column windows per batch. b1's taper lengthens toward the end:
# the final ~1 MB of output otherwise all evicts within the last ~1.8 us
# and its stores pile up past the last matmul.
WINSPLIT = {0: [256, 256, 256, 512, 512, 512, 512, 512, 512, 256],
            1: [512, 512, 512, 512, 512, 512, 512, 256, 256]}
# Output store groups per batch: (window-count, store spec). Store spec
# is an engine name for one strided 4-M-tile DMA, or a 4-tuple of engine
# names for per-M-tile stores that each chase their own eviction (used
# for the last windows so the final HBM writes hug the last matmuls).
# Concurrent queues round-robin per packet and these late stores have
# 1 KB lines (~140 GB/s per queue), so the last three windows spread
# per-M-tile over scalar+gpsimd, keeping sync's queue empty for w8.
OG_PLAN = {
    0: [(7, "gpsimd"), (3, "gpsimd")],
    1: [
        (4, "gpsimd"),
        (1, "gpsimd"),
        (1, "sync"),
        (1, ("scalar", "gpsimd", "scalar", "gpsimd")),
        (1, ("scalar", "gpsimd", "scalar", "gpsimd")),
        (1, ("sync", "scalar", "scalar", "sync")),
    ],
}

# HAM warmup matmuls. N=512 so each dummy streams the full 427 ns
# issue-to-issue gap (100% PE duty at 1.2 GHz) - N=256 dummies are only
# 50% duty and the HAM busy-window never fires on them (measured).
NDUMMY = 9
NDUM_N = 512

LAST_RESULT = None  # BassKernelResults of the most recent run (for test harness)
_cache = {}


def _windows(b):
    """Map WINSPLIT column windows onto CHUNKS -> (chunk_idx, off, ncols)."""
    ws = []
    ci, coff = 0, 0
    for n in WINSPLIT[b]:
        if coff >= CHUNKS[b][ci]:
            ci, coff = ci + 1, 0
        assert coff + n <= CHUNKS[b][ci]
        ws.append((ci, coff, n))
        coff += n
    assert sum(w[2] for w in ws) == T
    return ws


WINDOWS = {b: _windows(b) for b in range(BPC)}


def _build():
    # Bacc (not plain Bass): its finalize() runs move_matmul_waits_to_ldweights +
    # generate_event_semaphores, which walrus needs.
    nc = bacc.Bacc("TRN2", target_bir_lowering=False, debug=False, num_devices=NCORES)
    # Host layout: prex[b, p, 4*off + kt*ccols + j] for chunk at column off -
    # each chunk is a contiguous [128, 4*ccols] block (128 descriptors).
    prex = nc.dram_tensor("prex", [BPC, P, KT * T], X_DT, kind="ExternalInput").ap()
    # Host layout: wq[mt, p, kt*128 + m] = W[mt*128+m, kt*128+p] - one
    # [128, 1 KB] contiguous block per M-tile.
    wq = nc.dram_tensor("wq", [MT, P, KT * P], W_DT, kind="ExternalInput").ap()
    b2 = nc.dram_tensor("b2", [P, MT], mybir.dt.float32, kind="ExternalInput").ap()
    out = nc.dram_tensor("out", [BPC, C, T], OUT_DT, kind="ExternalOutput").ap()

    with ExitStack() as ctx:
        # HAM warmup scratch: a RAW sbuf tensor (not a tile-pool tile), read
        # uninitialized, so the dummy matmuls carry no memset dependency and
        # issue the moment the tensor engine clears its preamble (~6.2 us) -
        # the memset route cost ~1.3 us of event-semaphore latency.
        dummy = ctx.enter_context(nc.sbuf_tensor("ham_scratch", [P, NDUM_N], X_DT))
        tc = ctx.enter_context(tile.TileContext(nc))
        wpool = ctx.enter_context(tc.tile_pool(name="w", bufs=1))
        bpool = ctx.enter_context(tc.tile_pool(name="bias", bufs=1))
        xpool = ctx.enter_context(tc.tile_pool(name="x", bufs=1))
        opool = ctx.enter_context(tc.tile_pool(name="o", bufs=1))
        pspool = ctx.enter_context(tc.tile_pool(name="ps", bufs=7, space="PSUM"))

        for i in range(NDUMMY):
            # Rotate through the main psum bufs: a single dedicated buffer
            # WAW-serializes the dummies (768 ns each instead of 427).
            ps = pspool.tile([P, NDUM_N], mybir.dt.float32, tag="ps", name=f"psd_{i}")
            nc.tensor.matmul(ps[:], dummy[:, 0:P], dummy[:, :], start=True, stop=True)

        wtiles = {}
        xtiles = {}

        def load_x(b, ci, off):
            cols = CHUNKS[b][ci]
            x = xpool.tile([P, KT, cols], X_DT, name=f"x_{b}_{ci}", tag=f"x_{b}_{ci}")
            nc.scalar.dma_start(x[:], prex[b, :, bass.ds(KT * off, KT * cols)])
            xtiles[b, ci] = x

        def load_w(mt):
            w = wpool.tile([P, KT * P], W_DT, name=f"w_{mt}", tag=f"w_{mt}")
            nc.scalar.dma_start(w[:], wq[mt])
            wtiles[mt] = w

        # The whole input in consumption order on the scalar queue.
        offs0 = np.cumsum([0] + CHUNKS[0]).tolist()
        offs1 = np.cumsum([0] + CHUNKS[1]).tolist()
        load_w(0)
        load_x(0, 0, offs0[0])
        btile = bpool.tile([P, MT], mybir.dt.float32)
        nc.scalar.dma_start(btile[:], b2[:])
        for mt in range(1, MT):
            load_w(mt)
        for ci in range(1, len(CHUNKS[0])):
            load_x(0, ci, offs0[ci])
        for ci in range(len(CHUNKS[1])):
            load_x(1, ci, offs1[ci])

        def wslice(kt, mt):
            return wtiles[mt][:, kt * P : (kt + 1) * P]

        engs = {"sync": nc.sync, "scalar": nc.scalar, "gpsimd": nc.gpsimd}
        for b in range(BPC):
            wins = WINDOWS[b]
            wi = 0
            obase = 0
            for og, (nwin, store) in enumerate(OG_PLAN[b]):
                ws = wins[wi : wi + nwin]
                ocols = sum(w[2] for w in ws)
                otile = opool.tile(
                    [P, MT, ocols], OUT_DT, name=f"o_{b}_{og}", tag=f"o_{b}_{og}"
                )
                per_mt = not isinstance(store, str)
                woffs = np.cumsum([0] + [w[2] for w in ws]).tolist()
                for iw, (ci, xoff, ncols) in enumerate(ws):
                    for mt in range(MT):
                        ps = pspool.tile([P, ncols], mybir.dt.float32, tag="ps")
                        for kt in range(KT):
                            nc.tensor.matmul(
                                ps[:],
                                wslice(kt, mt),
                                xtiles[b, ci][:, kt, xoff : xoff + ncols],
                                start=(kt == 0),
                                stop=(kt == KT - 1),
                            )
                        # x is pre-scaled by 2 on the host, so only + 2*bias
                        # remains. All evictions on DVE: scalar is busy
                        # desc-genning the input queue in the head phase.
                        dst = otile[:, mt, woffs[iw] : woffs[iw] + ncols]
                        nc.vector.tensor_scalar_add(dst, ps[:], btile[:, mt : mt + 1])
                        if per_mt and iw == nwin - 1:
                            engs[store[mt]].dma_start(
                                out[b, mt * P : (mt + 1) * P, bass.ds(obase, ocols)],
                                otile[:, mt, :],
                            )
                if not per_mt:
                    # One strided store covers all 4 M-tiles.
                    dst_d = out[b, :, bass.ds(obase, ocols)].rearrange(
                        "(mt p) j -> p mt j", mt=MT
                    )
                    engs[store].dma_start(dst_d, otile[:])
                wi += nwin
                obase += ocols
    # The axon/PJRT exec path serializes nc as-is; finalize here so Bacc's
    # compile passes (register alloc, event-semaphore wait splitting) run.
    nc.finalize()
    return nc


def _blocked_x(pre8):
    """[B, C, T] e3m4 -> [B, P, KT*T]: per chunk, [p, kt, j] contiguous."""
    out = np.empty((B, P, KT * T), dtype=X_NP)
    for b in range(B):
        off = 0
        for cols in CHUNKS[b % BPC]:
            blk = pre8[b, :, off : off + cols].reshape(KT, P, cols)
            out[b, :, KT * off : KT * (off + cols)] = blk.transpose(1, 0, 2).reshape(
                P, KT * cols
            )
            off += cols
    return out


def kernel(pre, W_pre, b_pre):
    global LAST_RESULT
    # Fold the reference's final y+y into x and bias: out = W@(2x) + 2b.
    pre8 = (2.0 * np.asarray(pre, dtype=np.float32)).astype(X_NP)
    prex = _blocked_x(pre8)
    w = np.asarray(W_pre, dtype=np.float32)
    wq = np.ascontiguousarray(
        w.reshape(MT, P, KT, P).transpose(0, 3, 2, 1).reshape(MT, P, KT * P)
    ).astype(np.float16)
    b2 = np.ascontiguousarray(
        (2.0 * np.asarray(b_pre, dtype=np.float32)).reshape(MT, P).T
    )
    if "nc" not in _cache:
        _cache["nc"] = _build()
    nc = _cache["nc"]
    in_maps = [
        {"prex": prex[i * BPC : (i + 1) * BPC], "wq": wq, "b2": b2}
        for i in range(NCORES)
    ]
    res = run_bass_kernel_spmd(nc, in_maps, list(range(NCORES)))
    LAST_RESULT = res
    return np.ascontiguousarray(
        np.concatenate([res.results[i]["out"] for i in range(NCORES)], axis=0)
    ).astype(np.float32)


# revision 34
# speedup vs baseline: 1.0031x; 1.0031x over previous
"""Trainium2 Bass kernel for y = 2*(einsum('bct,oc->bot', pre, W_pre) + b_pre).

Shapes (hardcoded): pre [16, 512, 4096] f32, W_pre [512, 512] f32, b_pre [512] f32.
Sharding: data-parallel over B across 8 cores (2 batches per core).

Per core: out[b, o, t] = sum_c W[o,c]*(2*pre)[b,c,t] + 2*bias[o] for 2 batches
(the reference's y+y is folded into x and bias on the host).

Precision/dtype plan (tolerance is max-rel-err < 2e-2 vs max|y|):
- moving operand x = fp8 E3M4 (float8e3): 4 mantissa bits. Host quantizes
  2*pre; max|2*pre| ~ 10.8 < 15.5 (e3m4 max normal), and the PE handles
  e3m4 subnormals exactly (verified by canary). Measured end-to-end rel
  err 1.61e-2. fp8e4 (needed for DoubleRow 2x PE) measures 4.2e-2 - fails
  - so the PE runs at the 1 row/cycle fp16-class rate and the kernel is
  PE-streaming-bound at ~55.3 us warm (131072 stream cycles @ 2.4 GHz).
- stationary W = fp16 (full precision for this data), output = fp16.
- fp8 x halves input DMA to 4.2 MB/core (out 8.4 MB fp16, W 0.5 MB):
  total ~13 MB << PE time at ~400 GB/s, so DMA fully hides.

Schedule notes (from NTFF traces):
- Engines can't start until their instruction iram loads (~53 KB @ ~16
  GB/s, done ~5.7 us) plus per-engine TENSOR_LOAD barriers; the scalar
  (ACT) engine is free first (~5.85 us), sync last (~6.6 us).
- SDMA round-robins *per packet* across active queues, so splitting the
  head loads over several queues delays every transfer's completion
  (measured: first matmul slipped 7 us). Instead the WHOLE input rides
  the scalar HWDGE queue in exact consumption order:
  W0, x b0c0, bias, W1, W2, W3, b0c1..c4, b1c0, b1c1 - and nothing else
  touches SDMA until the input is done (~20.5 us). Descriptor generation
  (~670 ns each) occupies the scalar engine until ~13.9 us, which is why
  ALL PSUM evictions run on DVE (66% busy warm - fits); scalar only
  desc-gens.
- PE HAM clock gate: 1.2 GHz cold, 2.4 GHz after ~3.4 us of sustained
  matmul activity. 8 dummy matmuls (N=256, scratch SBUF, own PSUM tag)
  bridge from ~6.2 us (memset lands on the gpsimd queue right after its
  preamble) to first-data at ~7.9 us.
- Matmul windows are >=256 columns: at N=128 the 107 ns LDWEIGHTS no
  longer hides behind the 56 ns stream and the PE becomes LDW-bound.
  Window plan per batch: b0 [256, 512x7, 256], b1 [512x7, 256, 256]
  (small first window so the first chunk lands early; small last
  windows so the final stores chase the last matmul closely).
- Output store groups ride the gpsimd SWDGE queue except b1's
  next-to-last (w7, 256 cols, on scalar - keeps sync's HWDGE queue
  empty) and the final w8 (256 cols), stored per-M-tile on
  sync/scalar/gpsimd/sync in parallel, each chasing its own DVE
  eviction, so the last HBM write is a 64 KB transfer ~1 us after the
  last matmul.
"""

import os
import sys

for _p in ("/opt/trn_rl_repo", "/root/.axon_site/_ro/trn_rl_repo"):
    if os.path.isdir(_p) and _p not in sys.path:
        sys.path.append(_p)

from contextlib import ExitStack

import ml_dtypes
import numpy as np

import concourse.bass as bass
import concourse.tile as tile
from concourse import bacc, mybir
from concourse.bass_utils import run_bass_kernel_spmd

B, C, T = 16, 512, 4096  # batch, channels (in == out), sequence
NCORES = 8
BPC = B // NCORES  # batches per core
P = 128
KT = C // P  # contraction tiles
MT = C // P  # output-channel tiles

X_DT = mybir.dt.float8e3  # moving operand + input DMA dtype (E3M4)
W_DT = mybir.dt.float16  # stationary operand
OUT_DT = mybir.dt.float16  # output SBUF + DRAM dtype
X_NP = ml_dtypes.float8_e3m4

# Input DMA chunks per batch (each chunk is one contiguous-line DMA).
# b0's chunk sizes track the SDMA ramp (~50 -> 420 GB/s over 9-15 us) so
# each chunk's completion stays ~1 us ahead of the PE's demand for it
# even with the observed +-1 us run-to-run ramp variance.
CHUNKS = {0: [256, 256, 256, 512, 512, 1024, 1280], 1: [2048, 2048]}
# Matmul column windows per batch. b1's taper lengthens toward the end:
# the final ~1 MB of output otherwise all evicts within the last ~1.8 us
# and its stores pile up past the last matmul.
WINSPLIT = {0: [256, 256, 256, 512, 512, 512, 512, 512, 512, 256],
            1: [512, 512, 512, 512, 512, 512, 512, 256, 256]}
# Output store groups per batch: (window-count, store spec). Store spec
# is an engine name for one strided 4-M-tile DMA, or a 4-tuple of engine
# names for per-M-tile stores that each chase their own eviction (used
# for the last windows so the final HBM writes hug the last matmuls).
# Concurrent queues round-robin per packet and these late stores have
# 1 KB lines (~140 GB/s per queue), so the last three windows spread
# per-M-tile over scalar+gpsimd, keeping sync's queue empty for w8.
OG_PLAN = {
    0: [(7, "gpsimd"), (3, "gpsimd")],
    1: [
        (4, "gpsimd"),
        (1, "gpsimd"),
        (1, "sync"),
        (1, ("scalar", "gpsimd", "scalar", "gpsimd")),
        (1, ("scalar", "gpsimd", "scalar", "gpsimd")),
        (1, ("sync", "scalar", "gpsimd", "sync")),
    ],
}

# HAM warmup matmuls. N=512 so each dummy streams the full 427 ns
# issue-to-issue gap (100% PE duty at 1.2 GHz) - N=256 dummies are only
# 50% duty and the HAM busy-window never fires on them (measured).
NDUMMY = 9
NDUM_N = 512

LAST_RESULT = None  # BassKernelResults of the most recent run (for test harness)
_cache = {}


def _windows(b):
    """Map WINSPLIT column windows onto CHUNKS -> (chunk_idx, off, ncols)."""
    ws = []
    ci, coff = 0, 0
    for n in WINSPLIT[b]:
        if coff >= CHUNKS[b][ci]:
            ci, coff = ci + 1, 0
        assert coff + n <= CHUNKS[b][ci]
        ws.append((ci, coff, n))
        coff += n
    assert sum(w[2] for w in ws) == T
    return ws


WINDOWS = {b: _windows(b) for b in range(BPC)}


def _build():
    # Bacc (not plain Bass): its finalize() runs move_matmul_waits_to_ldweights +
    # generate_event_semaphores, which walrus needs.
    nc = bacc.Bacc("TRN2", target_bir_lowering=False, debug=False, num_devices=NCORES)
    # Host layout: prex[b, p, 4*off + kt*ccols + j] for chunk at column off -
    # each chunk is a contiguous [128, 4*ccols] block (128 descriptors).
    prex = nc.dram_tensor("prex", [BPC, P, KT * T], X_DT, kind="ExternalInput").ap()
    # Host layout: wq[mt, p, kt*128 + m] = W[mt*128+m, kt*128+p] - one
    # [128, 1 KB] contiguous block per M-tile.
    wq = nc.dram_tensor("wq", [MT, P, KT * P], W_DT, kind="ExternalInput").ap()
    b2 = nc.dram_tensor("b2", [P, MT], mybir.dt.float32, kind="ExternalInput").ap()
    out = nc.dram_tensor("out", [BPC, C, T], OUT_DT, kind="ExternalOutput").ap()

    with ExitStack() as ctx:
        # HAM warmup scratch: a RAW sbuf tensor (not a tile-pool tile), read
        # uninitialized, so the dummy matmuls carry no memset dependency and
        # issue the moment the tensor engine clears its preamble (~6.2 us) -
        # the memset route cost ~1.3 us of event-semaphore latency.
        dummy = ctx.enter_context(nc.sbuf_tensor("ham_scratch", [P, NDUM_N], X_DT))
        tc = ctx.enter_context(tile.TileContext(nc))
        wpool = ctx.enter_context(tc.tile_pool(name="w", bufs=1))
        bpool = ctx.enter_context(tc.tile_pool(name="bias", bufs=1))
        xpool = ctx.enter_context(tc.tile_pool(name="x", bufs=1))
        opool = ctx.enter_context(tc.tile_pool(name="o", bufs=1))
        pspool = ctx.enter_context(tc.tile_pool(name="ps", bufs=7, space="PSUM"))

        for i in range(NDUMMY):
            # Rotate through the main psum bufs: a single dedicated buffer
            # WAW-serializes the dummies (768 ns each instead of 427).
            ps = pspool.tile([P, NDUM_N], mybir.dt.float32, tag="ps", name=f"psd_{i}")
            nc.tensor.matmul(ps[:], dummy[:, 0:P], dummy[:, :], start=True, stop=True)

        wtiles = {}
        xtiles = {}

        def load_x(b, ci, off):
            cols = CHUNKS[b][ci]
            x = xpool.tile([P, KT, cols], X_DT, name=f"x_{b}_{ci}", tag=f"x_{b}_{ci}")
            nc.scalar.dma_start(x[:], prex[b, :, bass.ds(KT * off, KT * cols)])
            xtiles[b, ci] = x

        def load_w(mt):
            w = wpool.tile([P, KT * P], W_DT, name=f"w_{mt}", tag=f"w_{mt}")
            nc.scalar.dma_start(w[:], wq[mt])
            wtiles[mt] = w

        # The whole input in consumption order on the scalar queue.
        offs0 = np.cumsum([0] + CHUNKS[0]).tolist()
        offs1 = np.cumsum([0] + CHUNKS[1]).tolist()
        load_w(0)
        load_x(0, 0, offs0[0])
        btile = bpool.tile([P, MT], mybir.dt.float32)
        nc.scalar.dma_start(btile[:], b2[:])
        for mt in range(1, MT):
            load_w(mt)
        for ci in range(1, len(CHUNKS[0])):
            load_x(0, ci, offs0[ci])
        for ci in range(len(CHUNKS[1])):
            load_x(1, ci, offs1[ci])

        def wslice(kt, mt):
            return wtiles[mt][:, kt * P : (kt + 1) * P]

        engs = {"sync": nc.sync, "scalar": nc.scalar, "gpsimd": nc.gpsimd}
        for b in range(BPC):
            wins = WINDOWS[b]
            wi = 0
            obase = 0
            for og, (nwin, store) in enumerate(OG_PLAN[b]):
                ws = wins[wi : wi + nwin]
                ocols = sum(w[2] for w in ws)
                otile = opool.tile(
                    [P, MT, ocols], OUT_DT, name=f"o_{b}_{og}", tag=f"o_{b}_{og}"
                )
                per_mt = not isinstance(store, str)
                woffs = np.cumsum([0] + [w[2] for w in ws]).tolist()
                for iw, (ci, xoff, ncols) in enumerate(ws):
                    for mt in range(MT):
                        ps = pspool.tile([P, ncols], mybir.dt.float32, tag="ps")
                        for kt in range(KT):
                            nc.tensor.matmul(
                                ps[:],
                                wslice(kt, mt),
                                xtiles[b, ci][:, kt, xoff : xoff + ncols],
                                start=(kt == 0),
                                stop=(kt == KT - 1),
                            )
                        # x is pre-scaled by 2 on the host, so only + 2*bias
                        # remains. All evictions on DVE: scalar is busy
                        # desc-genning the input queue in the head phase.
                        dst = otile[:, mt, woffs[iw] : woffs[iw] + ncols]
                        nc.vector.tensor_scalar_add(dst, ps[:], btile[:, mt : mt + 1])
                        if per_mt and iw == nwin - 1:
                            engs[store[mt]].dma_start(
                                out[b, mt * P : (mt + 1) * P, bass.ds(obase, ocols)],
                                otile[:, mt, :],
                            )
                if not per_mt:
                    # One strided store covers all 4 M-tiles.
                    dst_d = out[b, :, bass.ds(obase, ocols)].rearrange(
                        "(mt p) j -> p mt j", mt=MT
                    )
                    engs[store].dma_start(dst_d, otile[:])
                wi += nwin
                obase += ocols
    # The axon/PJRT exec path serializes nc as-is; finalize here so Bacc's
    # compile passes (register alloc, event-semaphore wait splitting) run.
    nc.finalize()
    return nc


def _blocked_x(pre8):
    """[B, C, T] e3m4 -> [B, P, KT*T]: per chunk, [p, kt, j] contiguous."""
    out = np.empty((B, P, KT * T), dtype=X_NP)
    for b in range(B):
        off = 0
        for cols in CHUNKS[b % BPC]:
            blk = pre8[b, :, off : off + cols].reshape(KT, P, cols)
            out[b, :, KT * off : KT * (off + cols)] = blk.transpose(1, 0, 2).reshape(
                P, KT * cols
            )
            off += cols
    return out


def kernel(pre, W_pre, b_pre):
    global LAST_RESULT
    # Fold the reference's final y+y into x and bias: out = W@(2x) + 2b.
    pre8 = (2.0 * np.asarray(pre, dtype=np.float32)).astype(X_NP)
    prex = _blocked_x(pre8)
    w = np.asarray(W_pre, dtype=np.float32)
    wq = np.ascontiguousarray(
        w.reshape(MT, P, KT, P).transpose(0, 3, 2, 1).reshape(MT, P, KT * P)
    ).astype(np.float16)
    b2 = np.ascontiguousarray(
        (2.0 * np.asarray(b_pre, dtype=np.float32)).reshape(MT, P).T
    )
    if "nc" not in _cache:
        _cache["nc"] = _build()
    nc = _cache["nc"]
    in_maps = [
        {"prex": prex[i * BPC : (i + 1) * BPC], "wq": wq, "b2": b2}
        for i in range(NCORES)
    ]
    res = run_bass_kernel_spmd(nc, in_maps, list(range(NCORES)))
    LAST_RESULT = res
    return np.ascontiguousarray(
        np.concatenate([res.results[i]["out"] for i in range(NCORES)], axis=0)
    ).astype(np.float32)
